# revision 10
# baseline (speedup 1.0000x reference)
"""AttnConv (GNN message passing) Trainium2 kernel — src-sharded edge-parallel.

Math: out[i] = sum_{e: dst_e=i} a_e * h[src_e], a = scatter-softmax(scores, dst),
scores = alpha_q[dst] + alpha_k[src] + b.  Within one dst group alpha_q[dst]+b
is constant and cancels in the softmax, so with w = exp(alpha_k - max alpha_k):
    out[i] = (sum_e w[src_e] * h[src_e]) / (sum_e w[src_e])

The axon host<->device tunnel moves ~35-70 MB/s, so bytes shipped per run
dominate; everything is laid out to minimize transfer:
 - Edges live on the core that owns their src row; each core gets only its
   1/8 slice of the gather table [w*h | w] (f16, 66-col rows).
 - dst space is tiled in 512-node blocks: each block gets
   ceil(max_core_edges/128) chunks of 128 edge slots (~12% padding); pad
   slots point at a zeroed table row so they contribute nothing.  Per-edge
   metadata is 3 bytes: uint16 packs the 14-bit local src index with the top
   2 bits of the 9-bit dst offset, uint8 carries the low 7 offset bits;
   decoded on device with shift/mask ops.
 - Per block the core gathers chunk rows (indirect DMA), builds one-hot
   [128 edges, 512 dst] masks (iota is_equal), and matmul-accumulates
   [65 features, 512 nodes] in PSUM (f32), flushing f16 into a feature-major
   partial [8 chunks, 65, 12800].
 - One f16 ReduceScatter(add) leaves each core its dst range; it divides
   num/den on device (f32 reciprocal broadcast across partitions via a
   contract-1 matmul) and returns f16 [64, 12800]; the host only assembles
   and casts.
 - The SPMD runner re-jits its wrapper every call; the persistent jax
   compilation cache turns the repeated XLA+BIR->NEFF compile into a ~0.1s
   lookup.
"""

import os

import numpy as np

import jax

try:
    jax.config.update(
        "jax_compilation_cache_dir",
        os.path.expanduser("~/.cache/jax-bass-cache"),
    )
    jax.config.update("jax_persistent_cache_min_entry_size_bytes", -1)
    jax.config.update("jax_persistent_cache_min_compile_time_secs", 0.0)
except Exception:
    pass

import concourse.bacc as bacc
import concourse.bass as bass
import concourse.tile as tile
from concourse import mybir
from concourse.bass_utils import run_bass_kernel_spmd

N_NODES = 100000
D = 64
N_CORES = 8
P = 128
NC_SRC = N_NODES // N_CORES          # 12500 table rows owned per core
NR = 12544                           # table rows padded (98 * 128)
PADROW = NR - 1                      # zeroed row used by pad slots
B5 = 512                             # dst nodes per block
NRC = 12800                          # dst nodes per core chunk (25 blocks)
NPAD = NRC * N_CORES                 # 102400 padded dst space
NBLK = NPAD // B5                    # 200 dst blocks
GW = 66                              # f16 table row: 64 w*h + w + pad
EW = 65                              # accumulated row: 64 w*h + w
GB = 2                               # dst blocks per work batch

F16 = mybir.dt.float16

last_results = None  # BassKernelResults of the most recent run (test harness)


def _preprocess(h, W_attn, edge_index):
    h = np.asarray(h, dtype=np.float32)
    W = np.asarray(W_attn, dtype=np.float32)
    src = np.asarray(edge_index[0]).astype(np.int64)
    dst = np.asarray(edge_index[1]).astype(np.int64)

    alpha = h @ W[D:, 0]
    w = np.exp(alpha - alpha.max(), dtype=np.float32)
    wh = h * w[:, None]
    gtab = np.zeros((N_CORES, NR, GW), dtype=np.float16)
    gtab[:, :NC_SRC, :D] = wh.astype(np.float16).reshape(N_CORES, NC_SRC, D)
    gtab[:, :NC_SRC, D] = w.astype(np.float16).reshape(N_CORES, NC_SRC)

    core = src // NC_SRC
    blk = dst >> 9
    off = dst & (B5 - 1)
    key = core * NBLK + blk
    order = np.lexsort((src, key))
    key_s = key[order]
    srcl_s = src[order] - core[order] * NC_SRC
    off_s = off[order]

    cnt = np.bincount(key_s, minlength=N_CORES * NBLK).reshape(N_CORES, NBLK)
    Kb = np.maximum(1, -(-cnt.max(axis=0) // P)).astype(np.int64)  # [NBLK]
    taskofs = np.zeros(NBLK + 1, dtype=np.int64)
    np.cumsum(Kb, out=taskofs[1:])
    M = int(taskofs[-1])

    cstart = np.zeros(N_CORES * NBLK, dtype=np.int64)
    np.cumsum(cnt.reshape(-1)[:-1], out=cstart[1:])
    rank = np.arange(key_s.shape[0], dtype=np.int64) - cstart[key_s]
    slot = (taskofs[key_s % NBLK] << 7) + rank
    core_s = key_s // NBLK

    aidx = np.full((N_CORES, M * P), PADROW, dtype=np.uint16)
    adst = np.zeros((N_CORES, M * P), dtype=np.uint8)
    aidx[core_s, slot] = (srcl_s | ((off_s >> 7) << 14)).astype(np.uint16)
    adst[core_s, slot] = (off_s & 127).astype(np.uint8)
    aidx = np.ascontiguousarray(aidx.reshape(N_CORES, M, P).transpose(0, 2, 1))
    adst = np.ascontiguousarray(adst.reshape(N_CORES, M, P).transpose(0, 2, 1))
    return gtab, aidx, adst, Kb, taskofs, M


def _build_program(M, Kb, taskofs):
    nc = bacc.Bacc(
        "TRN2",
        target_bir_lowering=False,
        debug=False,
        enable_asserts=False,
        num_devices=N_CORES,
    )
    gt = nc.dram_tensor("gtab", [NR, GW], F16, kind="ExternalInput")
    ai = nc.dram_tensor("aidx", [P, M], mybir.dt.uint16, kind="ExternalInput")
    ad = nc.dram_tensor("adst", [P, M], mybir.dt.uint8, kind="ExternalInput")
    outt = nc.dram_tensor("outt", [D, NRC], F16, kind="ExternalOutput")

    batches = []  # (b0, nb, t0, tb)
    for b0 in range(0, NBLK, GB):
        nb = min(GB, NBLK - b0)
        t0 = int(taskofs[b0])
        tb = int(taskofs[b0 + nb] - t0)
        batches.append((b0, nb, t0, tb))
    TBM = max(tb for _, _, _, tb in batches)

    with tile.TileContext(nc) as tc:
        with (
            tc.tile_pool(name="const", bufs=1) as cpool,
            tc.tile_pool(name="gath", bufs=3) as gpool,
            tc.tile_pool(name="oneh", bufs=3) as spool,
            tc.tile_pool(name="ob", bufs=4) as opool,
            tc.tile_pool(name="ps", bufs=6, space="PSUM") as pspool,
            tc.tile_pool(name="psb", bufs=2, space="PSUM") as pbpool,
            tc.tile_pool(name="dr", bufs=1, space="DRAM") as dpool,
        ):
            it16 = cpool.tile([P, B5], mybir.dt.int16)
            nc.gpsimd.iota(it16[:], pattern=[[1, B5]], channel_multiplier=0)
            it = cpool.tile([P, B5], F16)
            nc.vector.tensor_copy(out=it[:], in_=it16[:])
            ones = cpool.tile([1, D], F16)
            nc.vector.memset(ones[:], 1.0)

            u16 = cpool.tile([P, M], mybir.dt.uint16)
            nc.sync.dma_start(out=u16[:], in_=ai[:, :])
            u32 = cpool.tile([P, M], mybir.dt.int32)
            nc.vector.tensor_copy(out=u32[:], in_=u16[:])
            idx32 = cpool.tile([P, M], mybir.dt.int32)
            nc.vector.tensor_scalar(
                out=idx32[:],
                in0=u32[:],
                scalar1=16383,
                scalar2=None,
                op0=mybir.AluOpType.bitwise_and,
            )
            ad8 = cpool.tile([P, M], mybir.dt.uint8)
            nc.sync.dma_start(out=ad8[:], in_=ad[:, :])
            o32 = cpool.tile([P, M], mybir.dt.int32)
            nc.vector.tensor_copy(out=o32[:], in_=ad8[:])
            # off512 = (u >> 14) * 128 + low7  ==  ((u >> 14) << 7) | low7
            sh = cpool.tile([P, M], mybir.dt.int32)
            nc.vector.tensor_scalar(
                out=sh[:],
                in0=u32[:],
                scalar1=14,
                scalar2=7,
                op0=mybir.AluOpType.logical_shift_right,
                op1=mybir.AluOpType.logical_shift_left,
            )
            nc.vector.tensor_tensor(
                out=o32[:], in0=o32[:], in1=sh[:], op=mybir.AluOpType.add
            )
            adf = cpool.tile([P, M], F16)
            nc.vector.tensor_copy(out=adf[:], in_=o32[:])

            partial = dpool.tile([N_CORES, EW, NRC], F16)
            rsout = dpool.tile([EW, NRC], F16)

            for b0, nb, t0, tb in batches:
                gtile = gpool.tile([P, TBM * GW], F16, tag="gt")
                for k in range(tb):
                    nc.gpsimd.indirect_dma_start(
                        out=gtile[:, k * GW : (k + 1) * GW],
                        out_offset=None,
                        in_=gt[:, :],
                        in_offset=bass.IndirectOffsetOnAxis(
                            ap=idx32[:, t0 + k : t0 + k + 1], axis=0
                        ),
                    )
                sb = spool.tile([P, TBM * B5], F16, tag="oh")
                nc.any.tensor_tensor(
                    out=sb[:, 0 : tb * B5].rearrange("p (m q) -> p m q", q=B5),
                    in0=it[:].unsqueeze(1).to_broadcast([P, tb, B5]),
                    in1=adf[:, t0 : t0 + tb].unsqueeze(2).to_broadcast(
                        [P, tb, B5]
                    ),
                    op=mybir.AluOpType.is_equal,
                )
                for bi in range(nb):
                    b = b0 + bi
                    kb = int(Kb[b])
                    m0 = int(taskofs[b]) - t0
                    ps = pspool.tile([EW, B5], mybir.dt.float32, tag="ps")
                    for k in range(kb):
                        nc.tensor.matmul(
                            out=ps[:, :],
                            lhsT=gtile[:, (m0 + k) * GW : (m0 + k) * GW + EW],
                            rhs=sb[:, (m0 + k) * B5 : (m0 + k + 1) * B5],
                            start=(k == 0),
                            stop=(k == kb - 1),
                        )
                    ob = opool.tile([EW, B5], F16, tag="ob")
                    nc.scalar.copy(out=ob[:], in_=ps[:, :])
                    c5 = b // (NRC // B5)
                    col = (b % (NRC // B5)) * B5
                    nc.sync.dma_start(
                        out=partial[c5, :, col : col + B5], in_=ob[:]
                    )

            nc.gpsimd.collective_compute(
                "ReduceScatter",
                mybir.AluOpType.add,
                replica_groups=[list(range(N_CORES))],
                ins=[partial[:].opt()],
                outs=[rsout[:].opt()],
            )

            # out[:, j] = num[:, j] * (1/den[j]), broadcast across partitions
            # via a contract-1 matmul outer product
            for col in range(0, NRC, B5):
                t = opool.tile([EW, B5], F16, tag="dv")
                nc.sync.dma_start(out=t[:], in_=rsout[:, col : col + B5])
                den = opool.tile([1, B5], mybir.dt.float32, tag="dn")
                nc.vector.tensor_scalar(
                    out=den[:],
                    in0=t[D : D + 1, :],
                    scalar1=1e-12,
                    scalar2=None,
                    op0=mybir.AluOpType.max,
                )
                rec = opool.tile([1, B5], mybir.dt.float32, tag="rc")
                nc.vector.reciprocal(out=rec[:], in_=den[:])
                rec16 = opool.tile([1, B5], F16, tag="r6")
                nc.vector.tensor_copy(out=rec16[:], in_=rec[:])
                rb = pbpool.tile([D, B5], mybir.dt.float32, tag="rb")
                nc.tensor.matmul(
                    out=rb[:, :],
                    lhsT=ones[:],
                    rhs=rec16[:],
                    start=True,
                    stop=True,
                )
                of = opool.tile([D, B5], F16, tag="of")
                nc.vector.tensor_tensor(
                    out=of[:],
                    in0=t[0:D, :],
                    in1=rb[:, :],
                    op=mybir.AluOpType.mult,
                )
                nc.sync.dma_start(out=outt[:, col : col + B5], in_=of[:])
    nc.compile()
    return nc


def _run(h, h_attn_q, W_attn, b_attn, edge_index, **spmd_kwargs):
    global last_results
    import time as _time

    _t0 = _time.time()
    gtab, aidx, adst, Kb, taskofs, M = _preprocess(h, W_attn, edge_index)
    _t1 = _time.time()
    nc = _build_program(M, Kb, taskofs)
    _t2 = _time.time()
    in_maps = [
        {"gtab": gtab[c], "aidx": aidx[c], "adst": adst[c]}
        for c in range(N_CORES)
    ]
    res = run_bass_kernel_spmd(
        nc, in_maps, core_ids=list(range(N_CORES)), **spmd_kwargs
    )
    last_results = res
    _t3 = _time.time()
    if os.environ.get("GNN_PHASES"):
        print(
            f"[phases] preprocess {_t1 - _t0:.2f}s build+compile "
            f"{_t2 - _t1:.2f}s run1 {_t3 - _t2:.2f}s",
            flush=True,
        )

    if os.environ.get("GNN_TIME2"):
        global last_exec_s
        t0 = _time.time()
        res = run_bass_kernel_spmd(
            nc, in_maps, core_ids=list(range(N_CORES)), **spmd_kwargs
        )
        last_exec_s = _time.time() - t0
        last_results = res

    full = np.empty((D, NPAD), dtype=np.float16)
    for c in range(N_CORES):
        full[:, c * NRC : (c + 1) * NRC] = np.asarray(res.results[c]["outt"])
    return np.ascontiguousarray(full[:, :N_NODES].T).astype(np.float32)


def kernel(h, h_attn_q, W_attn, b_attn, edge_index):
    return _run(h, h_attn_q, W_attn, b_attn, edge_index)


# revision 11
# speedup vs baseline: 1.0754x; 1.0754x over previous
"""AttnConv (GNN message passing) Trainium2 kernel — src-sharded edge-parallel.

Math: out[i] = sum_{e: dst_e=i} a_e * h[src_e], a = scatter-softmax(scores, dst),
scores = alpha_q[dst] + alpha_k[src] + b.  Within one dst group alpha_q[dst]+b
is constant and cancels in the softmax, so with w = exp(alpha_k - max alpha_k):
    out[i] = (sum_e w[src_e] * h[src_e]) / (sum_e w[src_e])

The axon host<->device tunnel moves ~35-70 MB/s, so bytes shipped per run
dominate; everything is laid out to minimize transfer:
 - Edges live on the core that owns their src row; each core gets only its
   1/8 slice of the gather table [w*h | w] (f16, 66-col rows).
 - dst space is tiled in 512-node blocks: each block gets
   ceil(max_core_edges/128) chunks of 128 edge slots (~12% padding); pad
   slots point at a zeroed table row so they contribute nothing.  Per-edge
   metadata is 3 bytes: uint16 packs the 14-bit local src index with the top
   2 bits of the 9-bit dst offset, uint8 carries the low 7 offset bits;
   decoded on device with shift/mask ops.
 - Per block the core gathers chunk rows (indirect DMA), builds one-hot
   [128 edges, 512 dst] masks (iota is_equal), and matmul-accumulates
   [65 features, 512 nodes] in PSUM (f32), flushing f16 into a feature-major
   partial [8 chunks, 65, 12800].
 - One f16 ReduceScatter(add) leaves each core its dst range; it divides
   num/den on device (f32 reciprocal broadcast across partitions via a
   contract-1 matmul) and returns f16 [64, 12800]; the host only assembles
   and casts.
 - The SPMD runner re-jits its wrapper every call; the persistent jax
   compilation cache turns the repeated XLA+BIR->NEFF compile into a ~0.1s
   lookup.
"""

import os

import numpy as np

import jax

try:
    jax.config.update(
        "jax_compilation_cache_dir",
        os.path.expanduser("~/.cache/jax-bass-cache"),
    )
    jax.config.update("jax_persistent_cache_min_entry_size_bytes", -1)
    jax.config.update("jax_persistent_cache_min_compile_time_secs", 0.0)
except Exception:
    pass

import concourse.bacc as bacc
import concourse.bass as bass
import concourse.tile as tile
from concourse import mybir
from concourse.bass_utils import run_bass_kernel_spmd

N_NODES = 100000
D = 64
N_CORES = 8
P = 128
NC_SRC = N_NODES // N_CORES          # 12500 table rows owned per core
NR = 12544                           # table rows padded (98 * 128)
PADROW = NR - 1                      # zeroed row used by pad slots
B5 = 512                             # dst nodes per block
NRC = 12800                          # dst nodes per core chunk (25 blocks)
NPAD = NRC * N_CORES                 # 102400 padded dst space
NBLK = NPAD // B5                    # 200 dst blocks
GW = 66                              # f16 table row: 64 w*h + w + pad
EW = 65                              # accumulated row: 64 w*h + w
GB = 2                               # dst blocks per work batch

F16 = mybir.dt.float16

last_results = None  # BassKernelResults of the most recent run (test harness)


def _preprocess(h, W_attn, edge_index):
    h = np.asarray(h, dtype=np.float32)
    W = np.asarray(W_attn, dtype=np.float32)
    src = np.asarray(edge_index[0]).astype(np.int64)
    dst = np.asarray(edge_index[1]).astype(np.int64)

    alpha = h @ W[D:, 0]
    w = np.exp(alpha - alpha.max(), dtype=np.float32)
    wh = h * w[:, None]
    gtab = np.zeros((N_CORES, NR, GW), dtype=np.float16)
    gtab[:, :NC_SRC, :D] = wh.astype(np.float16).reshape(N_CORES, NC_SRC, D)
    gtab[:, :NC_SRC, D] = w.astype(np.float16).reshape(N_CORES, NC_SRC)

    core = src // NC_SRC
    blk = dst >> 9
    off = dst & (B5 - 1)
    key = core * NBLK + blk
    order = np.lexsort((src, key))
    key_s = key[order]
    srcl_s = src[order] - core[order] * NC_SRC
    off_s = off[order]

    cnt = np.bincount(key_s, minlength=N_CORES * NBLK).reshape(N_CORES, NBLK)
    Kb = np.maximum(1, -(-cnt.max(axis=0) // P)).astype(np.int64)  # [NBLK]
    taskofs = np.zeros(NBLK + 1, dtype=np.int64)
    np.cumsum(Kb, out=taskofs[1:])
    M = int(taskofs[-1])

    cstart = np.zeros(N_CORES * NBLK, dtype=np.int64)
    np.cumsum(cnt.reshape(-1)[:-1], out=cstart[1:])
    rank = np.arange(key_s.shape[0], dtype=np.int64) - cstart[key_s]
    slot = (taskofs[key_s % NBLK] << 7) + rank
    core_s = key_s // NBLK

    aidx = np.full((N_CORES, M * P), PADROW, dtype=np.uint16)
    adst = np.zeros((N_CORES, M * P), dtype=np.uint8)
    aidx[core_s, slot] = (srcl_s | ((off_s >> 7) << 14)).astype(np.uint16)
    adst[core_s, slot] = (off_s & 127).astype(np.uint8)
    aidx = np.ascontiguousarray(aidx.reshape(N_CORES, M, P).transpose(0, 2, 1))
    adst = np.ascontiguousarray(adst.reshape(N_CORES, M, P).transpose(0, 2, 1))
    return gtab, aidx, adst, Kb, taskofs, M


def _build_program(M, Kb, taskofs):
    nc = bacc.Bacc(
        "TRN2",
        target_bir_lowering=False,
        debug=False,
        enable_asserts=False,
        num_devices=N_CORES,
    )
    gt = nc.dram_tensor("gtab", [NR, GW], F16, kind="ExternalInput")
    ai = nc.dram_tensor("aidx", [P, M], mybir.dt.uint16, kind="ExternalInput")
    ad = nc.dram_tensor("adst", [P, M], mybir.dt.uint8, kind="ExternalInput")
    outt = nc.dram_tensor("outt", [D, NRC], F16, kind="ExternalOutput")

    batches = []  # (b0, nb, t0, tb)
    for b0 in range(0, NBLK, GB):
        nb = min(GB, NBLK - b0)
        t0 = int(taskofs[b0])
        tb = int(taskofs[b0 + nb] - t0)
        batches.append((b0, nb, t0, tb))
    TBM = max(tb for _, _, _, tb in batches)

    with tile.TileContext(nc) as tc:
        with (
            tc.tile_pool(name="const", bufs=1) as cpool,
            tc.tile_pool(name="gath", bufs=3) as gpool,
            tc.tile_pool(name="oneh", bufs=3) as spool,
            tc.tile_pool(name="ob", bufs=4) as opool,
            tc.tile_pool(name="ps", bufs=6, space="PSUM") as pspool,
            tc.tile_pool(name="psb", bufs=2, space="PSUM") as pbpool,
            tc.tile_pool(name="dr", bufs=1, space="DRAM") as dpool,
        ):
            it16 = cpool.tile([P, B5], mybir.dt.int16)
            nc.gpsimd.iota(it16[:], pattern=[[1, B5]], channel_multiplier=0)
            it = cpool.tile([P, B5], F16)
            nc.vector.tensor_copy(out=it[:], in_=it16[:])
            ones = cpool.tile([1, D], F16)
            nc.vector.memset(ones[:], 1.0)

            u16 = cpool.tile([P, M], mybir.dt.uint16)
            nc.sync.dma_start(out=u16[:], in_=ai[:, :])
            u32 = cpool.tile([P, M], mybir.dt.int32)
            nc.vector.tensor_copy(out=u32[:], in_=u16[:])
            idx32 = cpool.tile([P, M], mybir.dt.int32)
            nc.vector.tensor_scalar(
                out=idx32[:],
                in0=u32[:],
                scalar1=16383,
                scalar2=None,
                op0=mybir.AluOpType.bitwise_and,
            )
            ad8 = cpool.tile([P, M], mybir.dt.uint8)
            nc.sync.dma_start(out=ad8[:], in_=ad[:, :])
            o32 = cpool.tile([P, M], mybir.dt.int32)
            nc.vector.tensor_copy(out=o32[:], in_=ad8[:])
            # off512 = (u >> 14) * 128 + low7  ==  ((u >> 14) << 7) | low7
            sh = cpool.tile([P, M], mybir.dt.int32)
            nc.vector.tensor_scalar(
                out=sh[:],
                in0=u32[:],
                scalar1=14,
                scalar2=7,
                op0=mybir.AluOpType.logical_shift_right,
                op1=mybir.AluOpType.logical_shift_left,
            )
            nc.vector.tensor_tensor(
                out=o32[:], in0=o32[:], in1=sh[:], op=mybir.AluOpType.add
            )
            adf = cpool.tile([P, M], F16)
            nc.vector.tensor_copy(out=adf[:], in_=o32[:])

            partial = dpool.tile([N_CORES, EW, NRC], F16)
            rsout = dpool.tile([EW, NRC], F16)

            for b0, nb, t0, tb in batches:
                gtile = gpool.tile([P, TBM * GW], F16, tag="gt")
                for k in range(tb):
                    nc.gpsimd.indirect_dma_start(
                        out=gtile[:, k * GW : (k + 1) * GW],
                        out_offset=None,
                        in_=gt[:, :],
                        in_offset=bass.IndirectOffsetOnAxis(
                            ap=idx32[:, t0 + k : t0 + k + 1], axis=0
                        ),
                    )
                sb = spool.tile([P, TBM * B5], F16, tag="oh")
                nc.any.tensor_tensor(
                    out=sb[:, 0 : tb * B5].rearrange("p (m q) -> p m q", q=B5),
                    in0=it[:].unsqueeze(1).to_broadcast([P, tb, B5]),
                    in1=adf[:, t0 : t0 + tb].unsqueeze(2).to_broadcast(
                        [P, tb, B5]
                    ),
                    op=mybir.AluOpType.is_equal,
                )
                for bi in range(nb):
                    b = b0 + bi
                    kb = int(Kb[b])
                    m0 = int(taskofs[b]) - t0
                    ps = pspool.tile([EW, B5], mybir.dt.float32, tag="ps")
                    for k in range(kb):
                        nc.tensor.matmul(
                            out=ps[:, :],
                            lhsT=gtile[:, (m0 + k) * GW : (m0 + k) * GW + EW],
                            rhs=sb[:, (m0 + k) * B5 : (m0 + k + 1) * B5],
                            start=(k == 0),
                            stop=(k == kb - 1),
                        )
                    ob = opool.tile([EW, B5], F16, tag="ob")
                    nc.scalar.copy(out=ob[:], in_=ps[:, :])
                    c5 = b // (NRC // B5)
                    col = (b % (NRC // B5)) * B5
                    nc.sync.dma_start(
                        out=partial[c5, :, col : col + B5], in_=ob[:]
                    )

            nc.gpsimd.collective_compute(
                "ReduceScatter",
                mybir.AluOpType.add,
                replica_groups=[list(range(N_CORES))],
                ins=[partial[:].opt()],
                outs=[rsout[:].opt()],
            )

            # out[:, j] = num[:, j] * (1/den[j]), broadcast across partitions
            # via a contract-1 matmul outer product
            for col in range(0, NRC, B5):
                t = opool.tile([EW, B5], F16, tag="dv")
                nc.sync.dma_start(out=t[:], in_=rsout[:, col : col + B5])
                den = opool.tile([1, B5], mybir.dt.float32, tag="dn")
                # clamp so 1/den stays finite in f16 (zero-in-degree and
                # padded nodes have den=0; their num is 0 so out stays 0)
                nc.vector.tensor_scalar(
                    out=den[:],
                    in0=t[D : D + 1, :],
                    scalar1=2e-5,
                    scalar2=None,
                    op0=mybir.AluOpType.max,
                )
                rec = opool.tile([1, B5], mybir.dt.float32, tag="rc")
                nc.vector.reciprocal(out=rec[:], in_=den[:])
                rec16 = opool.tile([1, B5], F16, tag="r6")
                nc.vector.tensor_copy(out=rec16[:], in_=rec[:])
                rb = pbpool.tile([D, B5], mybir.dt.float32, tag="rb")
                nc.tensor.matmul(
                    out=rb[:, :],
                    lhsT=ones[:],
                    rhs=rec16[:],
                    start=True,
                    stop=True,
                )
                of = opool.tile([D, B5], F16, tag="of")
                nc.vector.tensor_tensor(
                    out=of[:],
                    in0=t[0:D, :],
                    in1=rb[:, :],
                    op=mybir.AluOpType.mult,
                )
                nc.sync.dma_start(out=outt[:, col : col + B5], in_=of[:])
    nc.compile()
    return nc


def _run(h, h_attn_q, W_attn, b_attn, edge_index, **spmd_kwargs):
    global last_results
    import time as _time

    _t0 = _time.time()
    gtab, aidx, adst, Kb, taskofs, M = _preprocess(h, W_attn, edge_index)
    _t1 = _time.time()
    nc = _build_program(M, Kb, taskofs)
    _t2 = _time.time()
    in_maps = [
        {"gtab": gtab[c], "aidx": aidx[c], "adst": adst[c]}
        for c in range(N_CORES)
    ]
    res = run_bass_kernel_spmd(
        nc, in_maps, core_ids=list(range(N_CORES)), **spmd_kwargs
    )
    last_results = res
    _t3 = _time.time()
    if os.environ.get("GNN_PHASES"):
        print(
            f"[phases] preprocess {_t1 - _t0:.2f}s build+compile "
            f"{_t2 - _t1:.2f}s run1 {_t3 - _t2:.2f}s",
            flush=True,
        )

    if os.environ.get("GNN_TIME2"):
        global last_exec_s
        t0 = _time.time()
        res = run_bass_kernel_spmd(
            nc, in_maps, core_ids=list(range(N_CORES)), **spmd_kwargs
        )
        last_exec_s = _time.time() - t0
        last_results = res

    full = np.empty((D, NPAD), dtype=np.float16)
    for c in range(N_CORES):
        full[:, c * NRC : (c + 1) * NRC] = np.asarray(res.results[c]["outt"])
    return np.ascontiguousarray(full[:, :N_NODES].T).astype(np.float32)


def kernel(h, h_attn_q, W_attn, b_attn, edge_index):
    return _run(h, h_attn_q, W_attn, b_attn, edge_index)


# revision 13
# speedup vs baseline: 1.0845x; 1.0085x over previous
"""AttnConv (GNN message passing) Trainium2 kernel — src-sharded edge-parallel.

Math: out[i] = sum_{e: dst_e=i} a_e * h[src_e], a = scatter-softmax(scores, dst),
scores = alpha_q[dst] + alpha_k[src] + b.  Within one dst group alpha_q[dst]+b
is constant and cancels in the softmax, so with w = exp(alpha_k - max alpha_k):
    out[i] = (sum_e w[src_e] * h[src_e]) / (sum_e w[src_e])

The axon host<->device tunnel moves ~35-70 MB/s, so bytes shipped per run
dominate; everything is laid out to minimize transfer:
 - Edges live on the core that owns their src row; each core gets only its
   1/8 slice of the gather table [w*h | w] (f16, 66-col rows).
 - dst space is tiled in 512-node blocks: each block gets
   ceil(max_core_edges/128) chunks of 128 edge slots (~12% padding); pad
   slots point at a zeroed table row so they contribute nothing.  Per-edge
   metadata is 3 bytes: uint16 packs the 14-bit local src index with the top
   2 bits of the 9-bit dst offset, uint8 carries the low 7 offset bits;
   decoded on device with shift/mask ops.
 - Per block the core gathers chunk rows (indirect DMA), builds one-hot
   [128 edges, 512 dst] masks (iota is_equal), and matmul-accumulates
   [65 features, 512 nodes] in PSUM (f32), flushing f16 into a feature-major
   partial [8 chunks, 65, 12800].
 - One f16 ReduceScatter(add) leaves each core its dst range; it divides
   num/den on device (f32 reciprocal broadcast across partitions via a
   contract-1 matmul) and returns f16 [64, 12800]; the host only assembles
   and casts.
 - The SPMD runner re-jits its wrapper every call; the persistent jax
   compilation cache turns the repeated XLA+BIR->NEFF compile into a ~0.1s
   lookup.
"""

import os

import numpy as np

import jax

try:
    jax.config.update(
        "jax_compilation_cache_dir",
        os.path.expanduser("~/.cache/jax-bass-cache"),
    )
    jax.config.update("jax_persistent_cache_min_entry_size_bytes", -1)
    jax.config.update("jax_persistent_cache_min_compile_time_secs", 0.0)
except Exception:
    pass

import concourse.bacc as bacc
import concourse.bass as bass
import concourse.tile as tile
from concourse import mybir
from concourse.bass_utils import run_bass_kernel_spmd

N_NODES = 100000
D = 64
N_CORES = 8
P = 128
NC_SRC = N_NODES // N_CORES          # 12500 table rows owned per core
NR = 12544                           # table rows padded (98 * 128)
PADROW = NR - 1                      # zeroed row used by pad slots
B5 = 512                             # dst nodes per block
NRC = 12800                          # dst nodes per core chunk (25 blocks)
NPAD = NRC * N_CORES                 # 102400 padded dst space
NBLK = NPAD // B5                    # 200 dst blocks
GW = 66                              # f16 table row: 64 w*h + w + pad
EW = 65                              # accumulated row: 64 w*h + w
GB = 2                               # dst blocks per work batch

F16 = mybir.dt.float16

last_results = None  # BassKernelResults of the most recent run (test harness)


def _preprocess(h, W_attn, edge_index):
    h = np.asarray(h, dtype=np.float32)
    W = np.asarray(W_attn, dtype=np.float32)
    src = np.asarray(edge_index[0]).astype(np.int64)
    dst = np.asarray(edge_index[1]).astype(np.int64)

    alpha = h @ W[D:, 0]
    w = np.exp(alpha - alpha.max(), dtype=np.float32)
    wh = h * w[:, None]
    gtab = np.zeros((N_CORES, NR, GW), dtype=np.float16)
    gtab[:, :NC_SRC, :D] = wh.astype(np.float16).reshape(N_CORES, NC_SRC, D)
    gtab[:, :NC_SRC, D] = w.astype(np.float16).reshape(N_CORES, NC_SRC)

    core = src // NC_SRC
    blk = dst >> 9
    off = dst & (B5 - 1)
    key = core * NBLK + blk
    order = np.lexsort((src, key))
    key_s = key[order]
    srcl_s = src[order] - core[order] * NC_SRC
    off_s = off[order]

    cnt = np.bincount(key_s, minlength=N_CORES * NBLK).reshape(N_CORES, NBLK)
    Kb = np.maximum(1, -(-cnt.max(axis=0) // P)).astype(np.int64)  # [NBLK]
    taskofs = np.zeros(NBLK + 1, dtype=np.int64)
    np.cumsum(Kb, out=taskofs[1:])
    M = int(taskofs[-1])

    cstart = np.zeros(N_CORES * NBLK, dtype=np.int64)
    np.cumsum(cnt.reshape(-1)[:-1], out=cstart[1:])
    rank = np.arange(key_s.shape[0], dtype=np.int64) - cstart[key_s]
    slot = (taskofs[key_s % NBLK] << 7) + rank
    core_s = key_s // NBLK

    aidx = np.full((N_CORES, M * P), PADROW, dtype=np.uint16)
    adst = np.zeros((N_CORES, M * P), dtype=np.uint8)
    aidx[core_s, slot] = (srcl_s | ((off_s >> 7) << 14)).astype(np.uint16)
    adst[core_s, slot] = (off_s & 127).astype(np.uint8)
    aidx = np.ascontiguousarray(aidx.reshape(N_CORES, M, P).transpose(0, 2, 1))
    adst = np.ascontiguousarray(adst.reshape(N_CORES, M, P).transpose(0, 2, 1))
    return gtab, aidx, adst, Kb, taskofs, M


def _build_program(M, Kb, taskofs):
    nc = bacc.Bacc(
        "TRN2",
        target_bir_lowering=False,
        debug=False,
        enable_asserts=False,
        num_devices=N_CORES,
    )
    gt = nc.dram_tensor("gtab", [NR, GW], F16, kind="ExternalInput")
    ai = nc.dram_tensor("aidx", [P, M], mybir.dt.uint16, kind="ExternalInput")
    ad = nc.dram_tensor("adst", [P, M], mybir.dt.uint8, kind="ExternalInput")
    outt = nc.dram_tensor("outt", [D, NRC], F16, kind="ExternalOutput")

    batches = []  # (b0, nb, t0, tb)
    for b0 in range(0, NBLK, GB):
        nb = min(GB, NBLK - b0)
        t0 = int(taskofs[b0])
        tb = int(taskofs[b0 + nb] - t0)
        batches.append((b0, nb, t0, tb))
    TBM = max(tb for _, _, _, tb in batches)

    with tile.TileContext(nc) as tc:
        with (
            tc.tile_pool(name="const", bufs=1) as cpool,
            tc.tile_pool(name="gath", bufs=3) as gpool,
            tc.tile_pool(name="oneh", bufs=3) as spool,
            tc.tile_pool(name="ob", bufs=4) as opool,
            tc.tile_pool(name="ps", bufs=6, space="PSUM") as pspool,
            tc.tile_pool(name="psb", bufs=2, space="PSUM") as pbpool,
            tc.tile_pool(name="dr", bufs=1, space="DRAM") as dpool,
        ):
            it16 = cpool.tile([P, B5], mybir.dt.int16)
            nc.gpsimd.iota(it16[:], pattern=[[1, B5]], channel_multiplier=0)
            it = cpool.tile([P, B5], F16)
            nc.vector.tensor_copy(out=it[:], in_=it16[:])
            ones = cpool.tile([1, D], F16)
            nc.vector.memset(ones[:], 1.0)

            u16 = cpool.tile([P, M], mybir.dt.uint16)
            nc.sync.dma_start(out=u16[:], in_=ai[:, :])
            u32 = cpool.tile([P, M], mybir.dt.int32)
            nc.vector.tensor_copy(out=u32[:], in_=u16[:])
            idx32 = cpool.tile([P, M], mybir.dt.int32)
            nc.vector.tensor_scalar(
                out=idx32[:],
                in0=u32[:],
                scalar1=16383,
                scalar2=None,
                op0=mybir.AluOpType.bitwise_and,
            )
            ad8 = cpool.tile([P, M], mybir.dt.uint8)
            nc.sync.dma_start(out=ad8[:], in_=ad[:, :])
            o32 = cpool.tile([P, M], mybir.dt.int32)
            nc.vector.tensor_copy(out=o32[:], in_=ad8[:])
            # off512 = (u >> 14) * 128 + low7  ==  ((u >> 14) << 7) | low7
            sh = cpool.tile([P, M], mybir.dt.int32)
            nc.vector.tensor_scalar(
                out=sh[:],
                in0=u32[:],
                scalar1=14,
                scalar2=7,
                op0=mybir.AluOpType.logical_shift_right,
                op1=mybir.AluOpType.logical_shift_left,
            )
            nc.vector.tensor_tensor(
                out=o32[:], in0=o32[:], in1=sh[:], op=mybir.AluOpType.add
            )
            adf = cpool.tile([P, M], F16)
            nc.vector.tensor_copy(out=adf[:], in_=o32[:])

            partial = dpool.tile([N_CORES, EW, NRC], F16)
            rsout = dpool.tile([EW, NRC], F16)

            for b0, nb, t0, tb in batches:
                gtile = gpool.tile([P, TBM * GW], F16, tag="gt")
                for k in range(tb):
                    nc.gpsimd.indirect_dma_start(
                        out=gtile[:, k * GW : (k + 1) * GW],
                        out_offset=None,
                        in_=gt[:, :],
                        in_offset=bass.IndirectOffsetOnAxis(
                            ap=idx32[:, t0 + k : t0 + k + 1], axis=0
                        ),
                    )
                sb = spool.tile([P, TBM * B5], F16, tag="oh")
                nc.any.tensor_tensor(
                    out=sb[:, 0 : tb * B5].rearrange("p (m q) -> p m q", q=B5),
                    in0=it[:].unsqueeze(1).to_broadcast([P, tb, B5]),
                    in1=adf[:, t0 : t0 + tb].unsqueeze(2).to_broadcast(
                        [P, tb, B5]
                    ),
                    op=mybir.AluOpType.is_equal,
                )
                for bi in range(nb):
                    b = b0 + bi
                    kb = int(Kb[b])
                    m0 = int(taskofs[b]) - t0
                    ps = pspool.tile([EW, B5], mybir.dt.float32, tag="ps")
                    for k in range(kb):
                        nc.tensor.matmul(
                            out=ps[:, :],
                            lhsT=gtile[:, (m0 + k) * GW : (m0 + k) * GW + EW],
                            rhs=sb[:, (m0 + k) * B5 : (m0 + k + 1) * B5],
                            start=(k == 0),
                            stop=(k == kb - 1),
                        )
                    ob = opool.tile([EW, B5], F16, tag="ob")
                    nc.scalar.copy(out=ob[:], in_=ps[:, :])
                    c5 = b // (NRC // B5)
                    col = (b % (NRC // B5)) * B5
                    nc.sync.dma_start(
                        out=partial[c5, :, col : col + B5], in_=ob[:]
                    )

            nc.gpsimd.collective_compute(
                "ReduceScatter",
                mybir.AluOpType.add,
                replica_groups=[list(range(N_CORES))],
                ins=[partial[:].opt()],
                outs=[rsout[:].opt()],
            )

            # out[:, j] = num[:, j] * (1/den[j]), broadcast across partitions
            # via a contract-1 matmul outer product
            for col in range(0, NRC, B5):
                t = opool.tile([EW, B5], F16, tag="dv")
                nc.sync.dma_start(out=t[:], in_=rsout[:, col : col + B5])
                den = opool.tile([1, B5], mybir.dt.float32, tag="dn")
                # clamp so 1/den stays finite in f16 (zero-in-degree and
                # padded nodes have den=0; their num is 0 so out stays 0)
                nc.vector.tensor_scalar(
                    out=den[:],
                    in0=t[D : D + 1, :],
                    scalar1=2e-5,
                    scalar2=None,
                    op0=mybir.AluOpType.max,
                )
                rec = opool.tile([1, B5], mybir.dt.float32, tag="rc")
                nc.vector.reciprocal(out=rec[:], in_=den[:])
                rec16 = opool.tile([1, B5], F16, tag="r6")
                nc.vector.tensor_copy(out=rec16[:], in_=rec[:])
                rb = pbpool.tile([D, B5], mybir.dt.float32, tag="rb")
                nc.tensor.matmul(
                    out=rb[:, :],
                    lhsT=ones[:],
                    rhs=rec16[:],
                    start=True,
                    stop=True,
                )
                of = opool.tile([D, B5], F16, tag="of")
                nc.vector.tensor_tensor(
                    out=of[:],
                    in0=t[0:D, :],
                    in1=rb[:, :],
                    op=mybir.AluOpType.mult,
                )
                nc.sync.dma_start(out=outt[:, col : col + B5], in_=of[:])
    nc.compile()
    return nc


def _run(h, h_attn_q, W_attn, b_attn, edge_index, **spmd_kwargs):
    global last_results
    import time as _time

    _t0 = _time.time()
    gtab, aidx, adst, Kb, taskofs, M = _preprocess(h, W_attn, edge_index)
    _t1 = _time.time()
    nc = _build_program(M, Kb, taskofs)
    _t2 = _time.time()
    in_maps = [
        {"gtab": gtab[c], "aidx": aidx[c], "adst": adst[c]}
        for c in range(N_CORES)
    ]
    res = run_bass_kernel_spmd(
        nc, in_maps, core_ids=list(range(N_CORES)), **spmd_kwargs
    )
    last_results = res
    _t3 = _time.time()
    if os.environ.get("GNN_PHASES"):
        print(
            f"[phases] preprocess {_t1 - _t0:.2f}s build+compile "
            f"{_t2 - _t1:.2f}s run1 {_t3 - _t2:.2f}s",
            flush=True,
        )

    if os.environ.get("GNN_TIME2"):
        global last_exec_s
        t0 = _time.time()
        res = run_bass_kernel_spmd(
            nc, in_maps, core_ids=list(range(N_CORES)), **spmd_kwargs
        )
        last_exec_s = _time.time() - t0
        last_results = res

    full = np.empty((D, NPAD), dtype=np.float16)
    for c in range(N_CORES):
        full[:, c * NRC : (c + 1) * NRC] = np.asarray(res.results[c]["outt"])
    return np.ascontiguousarray(full[:, :N_NODES].T).astype(np.float32)


def kernel(h, h_attn_q, W_attn, b_attn, edge_index):
    return _run(h, h_attn_q, W_attn, b_attn, edge_index)
